# revision 2
# baseline (speedup 1.0000x reference)
"""Trainium2 Bass kernel for nn_CausalCrossAttention (B=4, S=2048, C=1024, NH=16, HD=64).

Sharding: DP over batch (4) x TP over heads (2 groups of 8), 8 NeuronCores.
Core c handles batch b = c // 2, head group g = c % 2 (heads 8g..8g+7).

Per-core algorithm (all matmuls in float32r = tf32, fp32 accumulate):
  - Host pre-transposes x/y to [C, S] and permutes weights head-pair-interleaved
    (pair p rows 128p..128p+127; even head partitions 0..63, odd head 64..127).
  - K-projection -> resident Kt [feat, S]; V-projection -> resident Vx [S, 8*65]
    with a padding-mask column per head (col 64 of each 65-wide head block) so
    the PV matmul also emits the softmax denominator as its output row 64.
  - Q-projection is done just-in-time per (q-window, pair) into a small tile.
  - Scores computed transposed, S^T[k, q], two heads concurrently via PE
    row-groups (0,0)/(64,0); exp(s/8) on ScalarE writes P^T (f32r).  Causal:
    skip fully-masked k-tiles, shrink N for diagonal tiles, and multiply the
    single diagonal 128-col block by a constant triangular mask.
  - PV: psum[65, 512] += Vx_tile[k, 65].T @ P^T[k, q-window]; row 64 = denom.
  - Normalize: denom row -> SBUF (ScalarE copy), K=1 ones-matmul broadcasts it
    across 64 partitions in PSUM, approx-reciprocal on VectorE, fused multiply
    from PV psum into AOt [feat, S] (O-projection lhsT layout).
  - O-projection: out partial = AOt.T @ WoArr; host sums the two head-group
    partials and adds Wo_b.
"""

import numpy as np
from contextlib import ExitStack

import concourse.bacc as bacc
import concourse.mybir as mybir
import concourse.tile as tile
from concourse import bass_utils

F32 = mybir.dt.float32
F32R = mybir.dt.float32r
AF = mybir.ActivationFunctionType
OP = mybir.AluOpType

B, S, C, NH, HD = 4, 2048, 1024, 16, 64
N_CORES = 8


def _round_tf32(a: np.ndarray) -> np.ndarray:
    """Round fp32 -> tf32 (10-bit mantissa) with round-to-nearest-even."""
    a = np.ascontiguousarray(a, dtype=np.float32)
    u = a.view(np.uint32)
    lsb = (u >> np.uint32(13)) & np.uint32(1)
    r = (u + np.uint32(0x0FFF) + lsb) & np.uint32(0xFFFFE000)
    return r.view(np.float32)


def build_program(s_len, cin, pairs, hd=64, has_mask=False, has_qkb=False, has_vb=False):
    """Build + compile the per-core Bass program. Returns the Bacc object."""
    assert s_len % 512 == 0 and cin % 128 == 0 and hd == 64
    n_sw = s_len // 512          # 512-wide s windows
    n_ct = cin // 128            # cin contraction tiles
    n_st = s_len // 128          # 128-wide s tiles
    feat = 128 * pairs           # local feature dim (= 64 * local heads)
    n_h = 2 * pairs              # local heads
    vxw = n_h * 65               # Vx row width per k-tile
    cw_sizes = []
    rem = cin
    while rem > 0:
        cw_sizes.append(min(512, rem))
        rem -= 512

    nc = bacc.Bacc("TRN2", target_bir_lowering=False, debug=False,
                   num_devices=N_CORES)

    d_xT = nc.dram_tensor("xT", [cin, s_len], F32R, kind="ExternalInput")
    d_yT = nc.dram_tensor("yT", [cin, s_len], F32R, kind="ExternalInput")
    d_wq = nc.dram_tensor("wqT", [cin, feat], F32R, kind="ExternalInput")
    d_wk = nc.dram_tensor("wkT", [cin, feat], F32R, kind="ExternalInput")
    d_wv = nc.dram_tensor("wvT", [cin, feat], F32R, kind="ExternalInput")
    d_wo = nc.dram_tensor("woA", [feat, cin], F32R, kind="ExternalInput")
    d_tri = nc.dram_tensor("tri", [128, 128], F32, kind="ExternalInput")
    d_mvr = nc.dram_tensor("mvr", [s_len, n_h], F32R, kind="ExternalInput")
    d_ones = nc.dram_tensor("ones1", [1, 64], F32R, kind="ExternalInput")
    if has_qkb:
        d_qb = nc.dram_tensor("qb", [128, pairs], F32, kind="ExternalInput")
        d_kb = nc.dram_tensor("kb", [128, pairs], F32, kind="ExternalInput")
    if has_vb:
        d_vbr = nc.dram_tensor("vbr", [128, feat], F32, kind="ExternalInput")
    if has_mask:
        d_mvst = nc.dram_tensor("mvst", [128, n_st], F32, kind="ExternalInput")
    d_out = nc.dram_tensor("out", [s_len, cin], F32, kind="ExternalOutput")

    with tile.TileContext(nc) as tc, ExitStack() as ctx:
        cpool = ctx.enter_context(tc.tile_pool(name="const", bufs=1))
        wpool = ctx.enter_context(tc.tile_pool(name="wts", bufs=2))
        bigp = ctx.enter_context(tc.tile_pool(name="big", bufs=1))
        sbuf = ctx.enter_context(tc.tile_pool(name="stream", bufs=9))
        qtwp = ctx.enter_context(tc.tile_pool(name="qtw", bufs=3))
        ppool = ctx.enter_context(tc.tile_pool(name="pt", bufs=2))
        rpool = ctx.enter_context(tc.tile_pool(name="rp", bufs=2))
        dpool = ctx.enter_context(tc.tile_pool(name="dp", bufs=1))
        ps_pv = ctx.enter_context(tc.tile_pool(name="ps_pv", bufs=2, space="PSUM"))
        ps_s = ctx.enter_context(tc.tile_pool(name="ps_s", bufs=2, space="PSUM"))

        # constants
        tri = cpool.tile([128, 128], F32, tag="tri")
        nc.sync.dma_start(tri[:], d_tri[:])
        ones1 = cpool.tile([1, 64], F32R, tag="ones1")
        nc.sync.dma_start(ones1[:], d_ones[:])
        if has_qkb:
            qb = cpool.tile([128, pairs], F32, tag="qb")
            nc.sync.dma_start(qb[:], d_qb[:])
            kb = cpool.tile([128, pairs], F32, tag="kb")
            nc.sync.dma_start(kb[:], d_kb[:])
        if has_vb:
            vbr = cpool.tile([128, feat], F32, tag="vbr")
            nc.sync.dma_start(vbr[:], d_vbr[:])
        if has_mask:
            mvst = cpool.tile([128, n_st], F32, tag="mvst")
            nc.sync.dma_start(mvst[:], d_mvst[:])

        # K/V weights (shared 2-slot tag: wk,wv then wq,wo)
        wk = wpool.tile([128, n_ct * feat], F32R, tag="w")
        wv = wpool.tile([128, n_ct * feat], F32R, tag="w")
        for ct in range(n_ct):
            nc.sync.dma_start(wk[:, ct * feat:(ct + 1) * feat],
                              d_wk[ct * 128:(ct + 1) * 128, :])
            nc.sync.dma_start(wv[:, ct * feat:(ct + 1) * feat],
                              d_wv[ct * 128:(ct + 1) * 128, :])

        # big persistent tensors
        Kt = bigp.tile([128, pairs * s_len], F32R, tag="kt")
        Vx = bigp.tile([128, (s_len // 128) * vxw], F32R, tag="vx")

        # Vx denominator/mask columns
        for st in range(n_st):
            dst = Vx[:, st * vxw:(st + 1) * vxw].rearrange(
                "p (h c) -> p h c", c=65)[:, :, 64:65]
            src = d_mvr[st * 128:(st + 1) * 128, :].rearrange(
                "p (h c) -> p h c", c=1)
            nc.sync.dma_start(dst, src)

        pvtags = ["pvA", "pvB"]

        # ---- K and V projections (consume yT) ----
        for sw in range(n_sw):
            yts = []
            for ct in range(n_ct):
                t = sbuf.tile([128, 512], F32R, tag="s")
                nc.sync.dma_start(
                    t[:], d_yT[ct * 128:(ct + 1) * 128, sw * 512:(sw + 1) * 512])
                yts.append(t)
            for mt in range(pairs):
                pk = ps_pv.tile([128, 512], F32, tag=pvtags[mt % 2])
                for ct in range(n_ct):
                    nc.tensor.matmul(
                        pk[:],
                        wk[:, ct * feat + mt * 128: ct * feat + (mt + 1) * 128],
                        yts[ct][:],
                        start=(ct == 0), stop=(ct == n_ct - 1))
                dst = Kt[:, mt * s_len + sw * 512: mt * s_len + (sw + 1) * 512]
                if has_qkb:
                    nc.vector.tensor_scalar_add(dst, pk[:], kb[:, mt:mt + 1])
                else:
                    nc.vector.tensor_copy(dst, pk[:])
            for i in range(4):
                st = sw * 4 + i
                pvp = ps_pv.tile([128, feat], F32, tag=pvtags[i % 2])
                for ct in range(n_ct):
                    nc.tensor.matmul(
                        pvp[:], yts[ct][:, i * 128:(i + 1) * 128],
                        wv[:, ct * feat:(ct + 1) * feat],
                        start=(ct == 0), stop=(ct == n_ct - 1))
                dst = Vx[:, st * vxw:(st + 1) * vxw].rearrange(
                    "p (h c) -> p h c", c=65)[:, :, 0:64]
                src = pvp[:].rearrange("p (h c) -> p h c", c=64)
                if has_vb:
                    nc.vector.scalar_tensor_tensor(
                        out=dst, in0=src, scalar=0.0, in1=vbr[:].rearrange(
                            "p (h c) -> p h c", c=64),
                        op0=OP.bypass, op1=OP.add)
                    if has_mask:
                        nc.vector.tensor_scalar_mul(dst, dst, mvst[:, st:st + 1])
                elif has_mask:
                    nc.vector.tensor_scalar_mul(dst, src, mvst[:, st:st + 1])
                else:
                    nc.vector.tensor_copy(dst, src)

        # Q weights (reuses the released wk slot)
        wq = wpool.tile([128, n_ct * feat], F32R, tag="w")
        for ct in range(n_ct):
            nc.sync.dma_start(wq[:, ct * feat:(ct + 1) * feat],
                              d_wq[ct * 128:(ct + 1) * 128, :])

        # ---- attention (with JIT Q-projection), w-outer / pair-inner ----
        AOt = bigp.tile([128, pairs * s_len], F32R, tag="aot")
        for w in range(n_sw):
            q0 = w * 512
            xts = []
            for ct in range(n_ct):
                t = sbuf.tile([128, 512], F32R, tag="s")
                nc.sync.dma_start(
                    t[:], d_xT[ct * 128:(ct + 1) * 128, q0:q0 + 512])
                xts.append(t)
            for p in range(pairs):
                # JIT Q-projection for this (window, pair)
                pq = ps_s.tile([128, 512], F32, tag="sA")
                for ct in range(n_ct):
                    nc.tensor.matmul(
                        pq[:],
                        wq[:, ct * feat + p * 128: ct * feat + (p + 1) * 128],
                        xts[ct][:],
                        start=(ct == 0), stop=(ct == n_ct - 1))
                Qtw = qtwp.tile([128, 512], F32R, tag="q")
                if has_qkb:
                    nc.vector.tensor_scalar_add(Qtw[:], pq[:], qb[:, p:p + 1])
                else:
                    nc.vector.tensor_copy(Qtw[:], pq[:])

                nkt = 4 * (w + 1)
                pvA = ps_pv.tile([65, 512], F32, tag="pvA")
                pvB = ps_pv.tile([65, 512], F32, tag="pvB")
                for kt in range(nkt):
                    k0 = kt * 128
                    diag = kt >= 4 * w
                    c0 = (kt - 4 * w) * 128 if diag else 0
                    sA = ps_s.tile([128, 512], F32, tag="sA")
                    nc.tensor.matmul(
                        sA[:, c0:512],
                        Kt[0:64, p * s_len + k0: p * s_len + k0 + 128],
                        Qtw[0:64, c0:512],
                        start=True, stop=True, tile_position=(0, 0))
                    sB = ps_s.tile([128, 512], F32, tag="sB")
                    nc.tensor.matmul(
                        sB[:, c0:512],
                        Kt[64:128, p * s_len + k0: p * s_len + k0 + 128],
                        Qtw[64:128, c0:512],
                        start=True, stop=True, tile_position=(64, 0))
                    PtA = ppool.tile([128, 512], F32R, tag="PtA")
                    nc.scalar.activation(PtA[:, c0:512], sA[:, c0:512],
                                         AF.Exp, scale=0.125)
                    PtB = ppool.tile([128, 512], F32R, tag="PtB")
                    nc.scalar.activation(PtB[:, c0:512], sB[:, c0:512],
                                         AF.Exp, scale=0.125)
                    if diag:
                        nc.vector.tensor_tensor(
                            out=PtA[:, c0:c0 + 128], in0=PtA[:, c0:c0 + 128],
                            in1=tri[:], op=OP.mult)
                        nc.vector.tensor_tensor(
                            out=PtB[:, c0:c0 + 128], in0=PtB[:, c0:c0 + 128],
                            in1=tri[:], op=OP.mult)
                    nc.tensor.matmul(
                        pvA[:, c0:512],
                        Vx[:, kt * vxw + (2 * p) * 65: kt * vxw + (2 * p) * 65 + 65],
                        PtA[:, c0:512],
                        start=(kt == 0), stop=(kt == nkt - 1))
                    nc.tensor.matmul(
                        pvB[:, c0:512],
                        Vx[:, kt * vxw + (2 * p + 1) * 65: kt * vxw + (2 * p + 1) * 65 + 65],
                        PtB[:, c0:512],
                        start=(kt == 0), stop=(kt == nkt - 1))
                for idx, pv in ((0, pvA), (1, pvB)):
                    drow = dpool.tile([1, 512], F32R, tag=f"drow{idx}")
                    nc.scalar.activation(drow[:], pv[64:65, :], AF.Copy)
                    psD = ps_s.tile([64, 512], F32, tag=["sA", "sB"][idx])
                    nc.tensor.matmul(psD[:], ones1[:], drow[:],
                                     start=True, stop=True)
                    rsb = rpool.tile([64, 512], F32, tag=f"rsb{idx}")
                    nc.vector.reciprocal_approx_fast(out=rsb[:], in_=psD[:])
                    nc.vector.tensor_tensor(
                        out=AOt[64 * idx: 64 * idx + 64,
                                p * s_len + q0: p * s_len + q0 + 512],
                        in0=pv[0:64, :], in1=rsb[:], op=OP.mult)

        # ---- O projection ----
        wo = wpool.tile([128, pairs * cin], F32R, tag="w")
        for p4 in range(pairs):
            nc.sync.dma_start(wo[:, p4 * cin:(p4 + 1) * cin],
                              d_wo[p4 * 128:(p4 + 1) * 128, :])
        for st in range(n_st):
            co = 0
            for cwi, cw in enumerate(cw_sizes):
                po = ps_pv.tile([128, 512], F32, tag=pvtags[cwi % 2])
                for p4 in range(pairs):
                    nc.tensor.matmul(
                        po[:, 0:cw],
                        AOt[:, p4 * s_len + st * 128: p4 * s_len + (st + 1) * 128],
                        wo[:, p4 * cin + co: p4 * cin + co + cw],
                        start=(p4 == 0), stop=(p4 == pairs - 1))
                ot = rpool.tile([128, 512], F32, tag="osb")
                nc.vector.tensor_copy(ot[:, 0:cw], po[:, 0:cw])
                nc.sync.dma_start(
                    d_out[st * 128:(st + 1) * 128, co:co + cw], ot[:, 0:cw])
                co += cw

    nc.compile()
    return nc


_programs = {}


def _get_program(key):
    if key not in _programs:
        _programs[key] = build_program(S, C, 4, HD, *key)
    return _programs[key]


def make_core_inputs(x, y, mask, Wq_w, Wq_b, Wkv_w, Wkv_b, Wo_w,
                     s_len=S, cin=C, pairs=4, nh=NH):
    """Build the list of 8 per-core input dicts (host-side shard + permute)."""
    n_h = 2 * pairs
    feat = 128 * pairs
    has_mask = bool(np.any(mask))
    has_qkb = bool(np.any(Wq_b)) or bool(np.any(Wkv_b[:cin]))
    has_vb = bool(np.any(Wkv_b[cin:]))
    tri = np.triu(np.ones((128, 128), dtype=np.float32))
    ones1 = np.ones((1, 64), dtype=np.float32)
    n_st = s_len // 128

    in_maps = []
    for core in range(N_CORES):
        b = core // 2
        g = core % 2
        # feature permutation: col = 128*p + 64*half + d  <-  local head 2p+half
        cidx = np.arange(feat)
        pair_i = cidx // 128
        half = (cidx % 128) // 64
        d = cidx % 64
        qk_rows = (n_h * g + 2 * pair_i + half) * 64 + d
        v_rows = cin + (n_h * g + cidx // 64) * 64 + cidx % 64

        mvec = 1.0 - mask[b].astype(np.float32)
        m = {
            "xT": _round_tf32(x[b].T),
            "yT": _round_tf32(y[b].T),
            "wqT": _round_tf32(Wq_w[qk_rows, :].T),
            "wkT": _round_tf32(Wkv_w[qk_rows, :].T),
            "wvT": _round_tf32(Wkv_w[v_rows, :].T),
            "woA": _round_tf32(Wo_w[:, qk_rows].T),
            "tri": tri,
            "mvr": np.ascontiguousarray(
                np.repeat(mvec[:, None], n_h, axis=1)),
            "ones1": ones1,
        }
        if has_qkb:
            m["qb"] = np.ascontiguousarray(
                Wq_b[qk_rows].reshape(pairs, 128).T)
            m["kb"] = np.ascontiguousarray(
                Wkv_b[qk_rows].reshape(pairs, 128).T)
        if has_vb:
            m["vbr"] = np.tile(Wkv_b[v_rows][None, :], (128, 1))
        if has_mask:
            m["mvst"] = np.ascontiguousarray(
                mvec.reshape(n_st, 128).T)
        in_maps.append(m)
    return in_maps, (has_mask, has_qkb, has_vb)


def run(x, y, mask, Wq_w, Wq_b, Wkv_w, Wkv_b, Wo_w, Wo_b, trace=False):
    x = np.asarray(x, dtype=np.float32)
    y = np.asarray(y, dtype=np.float32)
    mask = np.asarray(mask)
    Wq_w = np.asarray(Wq_w, dtype=np.float32)
    Wq_b = np.asarray(Wq_b, dtype=np.float32)
    Wkv_w = np.asarray(Wkv_w, dtype=np.float32)
    Wkv_b = np.asarray(Wkv_b, dtype=np.float32)
    Wo_w = np.asarray(Wo_w, dtype=np.float32)
    Wo_b = np.asarray(Wo_b, dtype=np.float32)

    in_maps, flags = make_core_inputs(x, y, mask, Wq_w, Wq_b, Wkv_w, Wkv_b, Wo_w)
    nc = _get_program(flags)
    res = bass_utils.run_bass_kernel_spmd(
        nc, in_maps, core_ids=list(range(N_CORES)), trace=trace)
    out = np.empty((B, S, C), dtype=np.float32)
    for b in range(B):
        out[b] = res.results[2 * b]["out"] + res.results[2 * b + 1]["out"] + Wo_b
    return out, res


def kernel(x, y, mask, Wq_w, Wq_b, Wkv_w, Wkv_b, Wo_w, Wo_b):
    out, _ = run(x, y, mask, Wq_w, Wq_b, Wkv_w, Wkv_b, Wo_w, Wo_b, trace=False)
    return out


# revision 3
# speedup vs baseline: 1.2319x; 1.2319x over previous
"""Trainium2 Bass kernel for nn_CausalCrossAttention (B=4, S=2048, C=1024, NH=16, HD=64).

Sharding: DP over batch (4) x TP over heads (2 groups of 8), 8 NeuronCores.
Core c handles batch b = c // 2, head group g = c % 2 (heads 8g..8g+7).

Per-core algorithm (fp16 matmuls with fp32 PSUM accumulation; the final output
projection runs in float32r = tf32):
  - Host pre-transposes x/y to [C, S] and permutes weights head-pair-interleaved
    (pair p rows 128p..128p+127; even head partitions 0..63, odd head 64..127).
  - K-projection -> resident Kt [feat, S]; V-projection -> resident Vx [S, 8*65]
    with a padding-mask column per head (col 64 of each 65-wide head block) so
    the PV matmul also emits the softmax denominator as its output row 64.
  - Q-projection is done just-in-time per (q-window, pair) into a small tile.
  - Scores are computed transposed, S^T[k, q], two heads concurrently via PE
    row-groups (0,0)/(64,0) into ONE [128, 1024] two-bank PSUM tile; a single
    exp(s/8) activation covers both heads' score tiles and writes P^T (fp16).
    Causal: skip fully-masked k-tiles, shrink N for diagonal tiles, multiply
    the diagonal 128-col blocks by a constant triangular mask.
  - PV (software-pipelined one k-tile behind the scores so the PE never waits
    on the exp): psum[65, 512] += Vx_tile[k, 65].T @ P^T; row 64 = denominator.
  - Normalize: denom row -> SBUF (ScalarE copy), K=1 ones-matmul broadcasts it
    across 64 partitions in PSUM, approx-reciprocal on VectorE, fused multiply
    from PV psum into AOt [feat, S] (f32r, O-projection lhsT layout).
  - O-projection (interleaved after each q-window): out partial = AOt.T @ WoArr;
    host sums the two head-group partials and adds Wo_b.
"""

import numpy as np
from contextlib import ExitStack

import concourse.bacc as bacc
import concourse.mybir as mybir
import concourse.tile as tile
from concourse import bass_utils

F32 = mybir.dt.float32
F32R = mybir.dt.float32r
F16 = mybir.dt.float16
AF = mybir.ActivationFunctionType
OP = mybir.AluOpType

B, S, C, NH, HD = 4, 2048, 1024, 16, 64
N_CORES = 8


def _round_tf32(a: np.ndarray) -> np.ndarray:
    """Round fp32 -> tf32 (10-bit mantissa) with round-to-nearest-even."""
    a = np.ascontiguousarray(a, dtype=np.float32)
    u = a.view(np.uint32)
    lsb = (u >> np.uint32(13)) & np.uint32(1)
    r = (u + np.uint32(0x0FFF) + lsb) & np.uint32(0xFFFFE000)
    return r.view(np.float32)


def build_program(s_len, cin, pairs, hd=64, has_mask=False, has_qkb=False, has_vb=False):
    """Build + compile the per-core Bass program. Returns the Bacc object."""
    assert s_len % 512 == 0 and cin % 128 == 0 and hd == 64
    n_sw = s_len // 512          # 512-wide s windows
    n_ct = cin // 128            # cin contraction tiles
    n_st = s_len // 128          # 128-wide s tiles
    feat = 128 * pairs           # local feature dim (= 64 * local heads)
    n_h = 2 * pairs              # local heads
    vxw = n_h * 65               # Vx row width per k-tile
    cw_sizes = []
    rem = cin
    while rem > 0:
        cw_sizes.append(min(512, rem))
        rem -= 512

    nc = bacc.Bacc("TRN2", target_bir_lowering=False, debug=False,
                   num_devices=N_CORES)

    d_xT = nc.dram_tensor("xT", [cin, s_len], F16, kind="ExternalInput")
    d_yT = nc.dram_tensor("yT", [cin, s_len], F16, kind="ExternalInput")
    d_wq = nc.dram_tensor("wqT", [cin, feat], F16, kind="ExternalInput")
    d_wk = nc.dram_tensor("wkT", [cin, feat], F16, kind="ExternalInput")
    d_wv = nc.dram_tensor("wvT", [cin, feat], F16, kind="ExternalInput")
    d_wo = nc.dram_tensor("woA", [feat, cin], F32R, kind="ExternalInput")
    d_tri = nc.dram_tensor("tri", [128, 128], F32, kind="ExternalInput")
    d_mvr = nc.dram_tensor("mvr", [s_len, n_h], F16, kind="ExternalInput")
    d_ones = nc.dram_tensor("ones1", [1, 64], F32R, kind="ExternalInput")
    if has_qkb:
        d_qb = nc.dram_tensor("qb", [128, pairs], F32, kind="ExternalInput")
        d_kb = nc.dram_tensor("kb", [128, pairs], F32, kind="ExternalInput")
    if has_vb:
        d_vbr = nc.dram_tensor("vbr", [128, feat], F32, kind="ExternalInput")
    if has_mask:
        d_mvst = nc.dram_tensor("mvst", [128, n_st], F32, kind="ExternalInput")
    d_out = nc.dram_tensor("out", [s_len, cin], F32, kind="ExternalOutput")

    with tile.TileContext(nc) as tc, ExitStack() as ctx:
        cpool = ctx.enter_context(tc.tile_pool(name="const", bufs=1))
        wpool = ctx.enter_context(tc.tile_pool(name="wts", bufs=2))
        bigp = ctx.enter_context(tc.tile_pool(name="big", bufs=1))
        sbuf = ctx.enter_context(tc.tile_pool(name="stream", bufs=16))
        qtwp = ctx.enter_context(tc.tile_pool(name="qtw", bufs=3))
        ppool = ctx.enter_context(tc.tile_pool(name="pt", bufs=4))
        rpool = ctx.enter_context(tc.tile_pool(name="rp", bufs=2))
        dpool = ctx.enter_context(tc.tile_pool(name="dp", bufs=2))
        ps_pv = ctx.enter_context(tc.tile_pool(name="ps_pv", bufs=2, space="PSUM"))
        ps_s = ctx.enter_context(tc.tile_pool(name="ps_s", bufs=2, space="PSUM"))

        # K/V weights first (their DMAs gate the first projections)
        wk = wpool.tile([128, n_ct * feat], F16, tag="w")
        wv = wpool.tile([128, n_ct * feat], F16, tag="w")
        for ct in range(n_ct):
            nc.sync.dma_start(wk[:, ct * feat:(ct + 1) * feat],
                              d_wk[ct * 128:(ct + 1) * 128, :])
        for ct in range(n_ct):
            nc.sync.dma_start(wv[:, ct * feat:(ct + 1) * feat],
                              d_wv[ct * 128:(ct + 1) * 128, :])

        # constants
        tri = cpool.tile([128, 128], F32, tag="tri")
        nc.sync.dma_start(tri[:], d_tri[:])
        ones1 = cpool.tile([1, 64], F32R, tag="ones1")
        nc.sync.dma_start(ones1[:], d_ones[:])
        if has_qkb:
            qb = cpool.tile([128, pairs], F32, tag="qb")
            nc.sync.dma_start(qb[:], d_qb[:])
            kb = cpool.tile([128, pairs], F32, tag="kb")
            nc.sync.dma_start(kb[:], d_kb[:])
        if has_vb:
            vbr = cpool.tile([128, feat], F32, tag="vbr")
            nc.sync.dma_start(vbr[:], d_vbr[:])
        if has_mask:
            mvst = cpool.tile([128, n_st], F32, tag="mvst")
            nc.sync.dma_start(mvst[:], d_mvst[:])

        # big persistent tensors
        Kt = bigp.tile([128, pairs * s_len], F16, tag="kt")
        Vx = bigp.tile([128, (s_len // 128) * vxw], F16, tag="vx")
        AOt = bigp.tile([128, pairs * s_len], F32R, tag="aot")

        # Vx denominator/mask columns
        for st in range(n_st):
            dst = Vx[:, st * vxw:(st + 1) * vxw].rearrange(
                "p (h c) -> p h c", c=65)[:, :, 64:65]
            src = d_mvr[st * 128:(st + 1) * 128, :].rearrange(
                "p (h c) -> p h c", c=1)
            nc.sync.dma_start(dst, src)

        pvtags = ["pvA", "pvB"]

        # ---- K and V projections (consume yT) ----
        for sw in range(n_sw):
            yts = []
            for ct in range(n_ct):
                t = sbuf.tile([128, 512], F16, tag="s")
                nc.sync.dma_start(
                    t[:], d_yT[ct * 128:(ct + 1) * 128, sw * 512:(sw + 1) * 512])
                yts.append(t)
            for mt in range(pairs):
                pk = ps_pv.tile([128, 512], F32, tag=pvtags[mt % 2])
                for ct in range(n_ct):
                    nc.tensor.matmul(
                        pk[:],
                        wk[:, ct * feat + mt * 128: ct * feat + (mt + 1) * 128],
                        yts[ct][:],
                        start=(ct == 0), stop=(ct == n_ct - 1))
                dst = Kt[:, mt * s_len + sw * 512: mt * s_len + (sw + 1) * 512]
                if has_qkb:
                    nc.vector.tensor_scalar_add(dst, pk[:], kb[:, mt:mt + 1])
                else:
                    nc.vector.tensor_copy(dst, pk[:])
            for i in range(4):
                st = sw * 4 + i
                pvp = ps_pv.tile([128, feat], F32, tag=pvtags[i % 2])
                for ct in range(n_ct):
                    nc.tensor.matmul(
                        pvp[:], yts[ct][:, i * 128:(i + 1) * 128],
                        wv[:, ct * feat:(ct + 1) * feat],
                        start=(ct == 0), stop=(ct == n_ct - 1))
                dst = Vx[:, st * vxw:(st + 1) * vxw].rearrange(
                    "p (h c) -> p h c", c=65)[:, :, 0:64]
                src = pvp[:].rearrange("p (h c) -> p h c", c=64)
                if has_vb:
                    nc.vector.scalar_tensor_tensor(
                        out=dst, in0=src, scalar=0.0, in1=vbr[:].rearrange(
                            "p (h c) -> p h c", c=64),
                        op0=OP.bypass, op1=OP.add)
                    if has_mask:
                        nc.vector.tensor_scalar_mul(dst, dst, mvst[:, st:st + 1])
                elif has_mask:
                    nc.vector.tensor_scalar_mul(dst, src, mvst[:, st:st + 1])
                else:
                    nc.vector.tensor_copy(dst, src)

        # Q and O weights (reuse the released wk/wv slots)
        wq = wpool.tile([128, n_ct * feat], F16, tag="w")
        for ct in range(n_ct):
            nc.sync.dma_start(wq[:, ct * feat:(ct + 1) * feat],
                              d_wq[ct * 128:(ct + 1) * 128, :])
        wo = wpool.tile([128, pairs * cin], F32R, tag="w")
        for p4 in range(pairs):
            nc.sync.dma_start(wo[:, p4 * cin:(p4 + 1) * cin],
                              d_wo[p4 * 128:(p4 + 1) * 128, :])

        # ---- attention (JIT Q-projection, sAB combined exp, PV pipelined) ----
        for w in range(n_sw):
            q0 = w * 512
            xts = []
            for ct in range(n_ct):
                t = sbuf.tile([128, 512], F16, tag="s")
                nc.sync.dma_start(
                    t[:], d_xT[ct * 128:(ct + 1) * 128, q0:q0 + 512])
                xts.append(t)
            for p in range(pairs):
                # JIT Q-projection for this (window, pair)
                pq = ps_s.tile([128, 512], F32, tag="sA")
                for ct in range(n_ct):
                    nc.tensor.matmul(
                        pq[:],
                        wq[:, ct * feat + p * 128: ct * feat + (p + 1) * 128],
                        xts[ct][:],
                        start=(ct == 0), stop=(ct == n_ct - 1))
                Qtw = qtwp.tile([128, 512], F16, tag="q")
                if has_qkb:
                    nc.vector.tensor_scalar_add(Qtw[:], pq[:], qb[:, p:p + 1])
                else:
                    nc.vector.tensor_copy(Qtw[:], pq[:])

                nkt = 4 * (w + 1)
                pvA = ps_pv.tile([65, 512], F32, tag="pvA")
                pvB = ps_pv.tile([65, 512], F32, tag="pvB")
                pend = None  # (kt, c0, PtAB) awaiting its PV matmuls
                for kt in range(nkt):
                    k0 = kt * 128
                    diag = kt >= 4 * w
                    c0 = (kt - 4 * w) * 128 if diag else 0
                    sAB = ps_s.tile([128, 1024], F32, tag="sA")
                    nc.tensor.matmul(
                        sAB[:, c0:512],
                        Kt[0:64, p * s_len + k0: p * s_len + k0 + 128],
                        Qtw[0:64, c0:512],
                        start=True, stop=True, tile_position=(0, 0))
                    nc.tensor.matmul(
                        sAB[:, 512 + c0:1024],
                        Kt[64:128, p * s_len + k0: p * s_len + k0 + 128],
                        Qtw[64:128, c0:512],
                        start=True, stop=True, tile_position=(64, 0))
                    Pt = ppool.tile([128, 1024], F16, tag="Pt")
                    nc.scalar.activation(Pt[:, c0:1024], sAB[:, c0:1024],
                                         AF.Exp, scale=0.125)
                    if diag:
                        nc.vector.tensor_tensor(
                            out=Pt[:, c0:c0 + 128], in0=Pt[:, c0:c0 + 128],
                            in1=tri[:], op=OP.mult)
                        nc.vector.tensor_tensor(
                            out=Pt[:, 512 + c0:512 + c0 + 128],
                            in0=Pt[:, 512 + c0:512 + c0 + 128],
                            in1=tri[:], op=OP.mult)
                    if pend is not None:
                        pkt, pc0, pPt = pend
                        nc.tensor.matmul(
                            pvA[:, pc0:512],
                            Vx[:, pkt * vxw + (2 * p) * 65: pkt * vxw + (2 * p) * 65 + 65],
                            pPt[:, pc0:512],
                            start=(pkt == 0), stop=False)
                        nc.tensor.matmul(
                            pvB[:, pc0:512],
                            Vx[:, pkt * vxw + (2 * p + 1) * 65: pkt * vxw + (2 * p + 1) * 65 + 65],
                            pPt[:, 512 + pc0:1024],
                            start=(pkt == 0), stop=False)
                    pend = (kt, c0, Pt)
                pkt, pc0, pPt = pend
                nc.tensor.matmul(
                    pvA[:, pc0:512],
                    Vx[:, pkt * vxw + (2 * p) * 65: pkt * vxw + (2 * p) * 65 + 65],
                    pPt[:, pc0:512],
                    start=(pkt == 0), stop=True)
                nc.tensor.matmul(
                    pvB[:, pc0:512],
                    Vx[:, pkt * vxw + (2 * p + 1) * 65: pkt * vxw + (2 * p + 1) * 65 + 65],
                    pPt[:, 512 + pc0:1024],
                    start=(pkt == 0), stop=True)
                for idx, pv in ((0, pvA), (1, pvB)):
                    drow = dpool.tile([1, 512], F32R, tag=f"drow{idx}")
                    nc.scalar.activation(drow[:], pv[64:65, :], AF.Copy)
                    psD = ps_s.tile([64, 512], F32, tag="sA")
                    nc.tensor.matmul(psD[:], ones1[:], drow[:],
                                     start=True, stop=True)
                    rsb = rpool.tile([64, 512], F32, tag=f"rsb{idx}")
                    nc.vector.reciprocal_approx_fast(out=rsb[:], in_=psD[:])
                    nc.vector.tensor_tensor(
                        out=AOt[64 * idx: 64 * idx + 64,
                                p * s_len + q0: p * s_len + q0 + 512],
                        in0=pv[0:64, :], in1=rsb[:], op=OP.mult)

            # ---- O projection for this q-window (overlaps next window) ----
            for st in range(4 * w, 4 * w + 4):
                co = 0
                for cwi, cw in enumerate(cw_sizes):
                    po = ps_pv.tile([128, 512], F32, tag=pvtags[cwi % 2])
                    for p4 in range(pairs):
                        nc.tensor.matmul(
                            po[:, 0:cw],
                            AOt[:, p4 * s_len + st * 128: p4 * s_len + (st + 1) * 128],
                            wo[:, p4 * cin + co: p4 * cin + co + cw],
                            start=(p4 == 0), stop=(p4 == pairs - 1))
                    ot = rpool.tile([128, 512], F32, tag="osb")
                    nc.vector.tensor_copy(ot[:, 0:cw], po[:, 0:cw])
                    nc.sync.dma_start(
                        d_out[st * 128:(st + 1) * 128, co:co + cw], ot[:, 0:cw])
                    co += cw

    nc.compile()
    return nc


_programs = {}


def _get_program(key):
    if key not in _programs:
        _programs[key] = build_program(S, C, 4, HD, *key)
    return _programs[key]


def make_core_inputs(x, y, mask, Wq_w, Wq_b, Wkv_w, Wkv_b, Wo_w,
                     s_len=S, cin=C, pairs=4, nh=NH):
    """Build the list of 8 per-core input dicts (host-side shard + permute)."""
    n_h = 2 * pairs
    feat = 128 * pairs
    has_mask = bool(np.any(mask))
    has_qkb = bool(np.any(Wq_b)) or bool(np.any(Wkv_b[:cin]))
    has_vb = bool(np.any(Wkv_b[cin:]))
    tri = np.triu(np.ones((128, 128), dtype=np.float32))
    ones1 = np.ones((1, 64), dtype=np.float32)
    n_st = s_len // 128

    in_maps = []
    for core in range(N_CORES):
        b = core // 2
        g = core % 2
        # feature permutation: col = 128*p + 64*half + d  <-  local head 2p+half
        cidx = np.arange(feat)
        pair_i = cidx // 128
        half = (cidx % 128) // 64
        d = cidx % 64
        qk_rows = (n_h * g + 2 * pair_i + half) * 64 + d
        v_rows = cin + (n_h * g + cidx // 64) * 64 + cidx % 64

        mvec = 1.0 - mask[b].astype(np.float32)
        m = {
            "xT": np.ascontiguousarray(x[b].T).astype(np.float16),
            "yT": np.ascontiguousarray(y[b].T).astype(np.float16),
            "wqT": np.ascontiguousarray(Wq_w[qk_rows, :].T).astype(np.float16),
            "wkT": np.ascontiguousarray(Wkv_w[qk_rows, :].T).astype(np.float16),
            "wvT": np.ascontiguousarray(Wkv_w[v_rows, :].T).astype(np.float16),
            "woA": _round_tf32(Wo_w[:, qk_rows].T),
            "tri": tri,
            "mvr": np.ascontiguousarray(
                np.repeat(mvec[:, None], n_h, axis=1)).astype(np.float16),
            "ones1": ones1,
        }
        if has_qkb:
            m["qb"] = np.ascontiguousarray(
                Wq_b[qk_rows].reshape(pairs, 128).T)
            m["kb"] = np.ascontiguousarray(
                Wkv_b[qk_rows].reshape(pairs, 128).T)
        if has_vb:
            m["vbr"] = np.tile(Wkv_b[v_rows][None, :], (128, 1))
        if has_mask:
            m["mvst"] = np.ascontiguousarray(
                mvec.reshape(n_st, 128).T)
        in_maps.append(m)
    return in_maps, (has_mask, has_qkb, has_vb)


def run(x, y, mask, Wq_w, Wq_b, Wkv_w, Wkv_b, Wo_w, Wo_b, trace=False):
    x = np.asarray(x, dtype=np.float32)
    y = np.asarray(y, dtype=np.float32)
    mask = np.asarray(mask)
    Wq_w = np.asarray(Wq_w, dtype=np.float32)
    Wq_b = np.asarray(Wq_b, dtype=np.float32)
    Wkv_w = np.asarray(Wkv_w, dtype=np.float32)
    Wkv_b = np.asarray(Wkv_b, dtype=np.float32)
    Wo_w = np.asarray(Wo_w, dtype=np.float32)
    Wo_b = np.asarray(Wo_b, dtype=np.float32)

    in_maps, flags = make_core_inputs(x, y, mask, Wq_w, Wq_b, Wkv_w, Wkv_b, Wo_w)
    nc = _get_program(flags)
    res = bass_utils.run_bass_kernel_spmd(
        nc, in_maps, core_ids=list(range(N_CORES)), trace=trace)
    out = np.empty((B, S, C), dtype=np.float32)
    for b in range(B):
        out[b] = res.results[2 * b]["out"] + res.results[2 * b + 1]["out"] + Wo_b
    return out, res


def kernel(x, y, mask, Wq_w, Wq_b, Wkv_w, Wkv_b, Wo_w, Wo_b):
    out, _ = run(x, y, mask, Wq_w, Wq_b, Wkv_w, Wkv_b, Wo_w, Wo_b, trace=False)
    return out


# revision 6
# speedup vs baseline: 1.5199x; 1.2337x over previous
"""Trainium2 Bass kernel for nn_CausalCrossAttention (B=4, S=2048, C=1024, NH=16, HD=64).

Sharding: DP over batch (4) x TP over heads (2 groups of 8), 8 NeuronCores.
Core c handles batch b = c // 2, head group g = c % 2 (heads 8g..8g+7).

Per-core algorithm (fp16 matmuls with fp32 PSUM accumulation; the final output
projection runs in float32r = tf32):
  - Host pre-transposes x/y to [C, S] and permutes weights head-pair-interleaved
    (pair p rows 128p..128p+127; even head partitions 0..63, odd head 64..127).
  - K-projection -> resident Kt [feat, S]; V-projection -> resident Vx [S, 8*65]
    with a padding-mask column per head (col 64 of each 65-wide head block) so
    the PV matmul also emits the softmax denominator as its output row 64.
  - Q-projection is done just-in-time per (q-window, pair) into a small tile.
  - Scores are computed transposed, S^T[k, q], two heads concurrently via PE
    row-groups (0,0)/(64,0) into ONE [128, 1024] two-bank PSUM tile; a single
    exp(s/8) activation covers both heads' score tiles and writes P^T (fp16).
    Causal: skip fully-masked k-tiles, shrink N for diagonal tiles, multiply
    the diagonal 128-col blocks by a constant triangular mask.
  - PV (software-pipelined one k-tile behind the scores so the PE never waits
    on the exp): psum[65, 512] += Vx_tile[k, 65].T @ P^T; row 64 = denominator.
  - Normalize: denom row -> SBUF (ScalarE copy), K=1 ones-matmul broadcasts it
    across 64 partitions in PSUM, approx-reciprocal on VectorE, fused multiply
    from PV psum into AOt [feat, S] (f32r, O-projection lhsT layout).
  - O-projection (interleaved after each q-window): out partial = AOt.T @ WoArr;
    host sums the two head-group partials and adds Wo_b.
"""

import numpy as np
from contextlib import ExitStack

import concourse.bacc as bacc
import concourse.mybir as mybir
import concourse.tile as tile
from concourse import bass_utils

F32 = mybir.dt.float32
F32R = mybir.dt.float32r
F16 = mybir.dt.float16
AF = mybir.ActivationFunctionType
OP = mybir.AluOpType

B, S, C, NH, HD = 4, 2048, 1024, 16, 64
N_CORES = 8


def _round_tf32(a: np.ndarray) -> np.ndarray:
    """Round fp32 -> tf32 (10-bit mantissa) with round-to-nearest-even."""
    a = np.ascontiguousarray(a, dtype=np.float32)
    u = a.view(np.uint32)
    lsb = (u >> np.uint32(13)) & np.uint32(1)
    r = (u + np.uint32(0x0FFF) + lsb) & np.uint32(0xFFFFE000)
    return r.view(np.float32)


def build_program(s_len, cin, pairs, hd=64, has_mask=False, has_qkb=False, has_vb=False):
    """Build + compile the per-core Bass program. Returns the Bacc object."""
    assert s_len % 512 == 0 and cin % 128 == 0 and hd == 64
    n_sw = s_len // 512          # 512-wide s windows
    n_ct = cin // 128            # cin contraction tiles
    n_st = s_len // 128          # 128-wide s tiles
    feat = 128 * pairs           # local feature dim (= 64 * local heads)
    n_h = 2 * pairs              # local heads
    vxw = n_h * 65               # Vx row width per k-tile
    cw_sizes = []
    rem = cin
    while rem > 0:
        cw_sizes.append(min(512, rem))
        rem -= 512

    nc = bacc.Bacc("TRN2", target_bir_lowering=False, debug=False,
                   num_devices=N_CORES)

    d_xT = nc.dram_tensor("xT", [cin, s_len], F16, kind="ExternalInput")
    d_yT = nc.dram_tensor("yT", [cin, s_len], F16, kind="ExternalInput")
    d_wq = nc.dram_tensor("wqT", [cin, feat], F16, kind="ExternalInput")
    d_wk = nc.dram_tensor("wkT", [cin, feat], F16, kind="ExternalInput")
    d_wv = nc.dram_tensor("wvT", [cin, feat], F16, kind="ExternalInput")
    d_wo = nc.dram_tensor("woA", [feat, cin], F32R, kind="ExternalInput")
    d_tri = nc.dram_tensor("tri", [128, 128], F32, kind="ExternalInput")
    d_mvr = nc.dram_tensor("mvr", [s_len, n_h], F16, kind="ExternalInput")
    d_ones = nc.dram_tensor("ones1", [1, 64], F32R, kind="ExternalInput")
    if has_qkb:
        d_qb = nc.dram_tensor("qb", [128, pairs], F32, kind="ExternalInput")
        d_kb = nc.dram_tensor("kb", [128, pairs], F32, kind="ExternalInput")
    if has_vb:
        d_vbr = nc.dram_tensor("vbr", [128, feat], F32, kind="ExternalInput")
    if has_mask:
        d_mvst = nc.dram_tensor("mvst", [128, n_st], F32, kind="ExternalInput")
    d_out = nc.dram_tensor("out", [s_len, cin], F32, kind="ExternalOutput")

    with tile.TileContext(nc) as tc, ExitStack() as ctx:
        cpool = ctx.enter_context(tc.tile_pool(name="const", bufs=1))
        wpool = ctx.enter_context(tc.tile_pool(name="wts", bufs=3))
        bigp = ctx.enter_context(tc.tile_pool(name="big", bufs=1))
        sbuf = ctx.enter_context(tc.tile_pool(name="stream", bufs=16))
        qtwp = ctx.enter_context(tc.tile_pool(name="qtw", bufs=3))
        ppool = ctx.enter_context(tc.tile_pool(name="pt", bufs=4))
        rpool = ctx.enter_context(tc.tile_pool(name="rp", bufs=2))
        dpool = ctx.enter_context(tc.tile_pool(name="dp", bufs=2))
        ps_pv = ctx.enter_context(tc.tile_pool(name="ps_pv", bufs=2, space="PSUM"))
        ps_s = ctx.enter_context(tc.tile_pool(name="ps_s", bufs=2, space="PSUM"))

        # K/V/Q weights first (their DMAs gate the first projections)
        wk = wpool.tile([128, n_ct * feat], F16, tag="w")
        wv = wpool.tile([128, n_ct * feat], F16, tag="w")
        wq = wpool.tile([128, n_ct * feat], F16, tag="w")
        for ct in range(n_ct):
            nc.sync.dma_start(wk[:, ct * feat:(ct + 1) * feat],
                              d_wk[ct * 128:(ct + 1) * 128, :])
        for ct in range(n_ct):
            nc.sync.dma_start(wv[:, ct * feat:(ct + 1) * feat],
                              d_wv[ct * 128:(ct + 1) * 128, :])
        for ct in range(n_ct):
            nc.sync.dma_start(wq[:, ct * feat:(ct + 1) * feat],
                              d_wq[ct * 128:(ct + 1) * 128, :])

        # constants
        tri = cpool.tile([128, 128], F32, tag="tri")
        nc.sync.dma_start(tri[:], d_tri[:])
        ones1 = cpool.tile([1, 64], F32R, tag="ones1")
        nc.sync.dma_start(ones1[:], d_ones[:])
        if has_qkb:
            qb = cpool.tile([128, pairs], F32, tag="qb")
            nc.sync.dma_start(qb[:], d_qb[:])
            kb = cpool.tile([128, pairs], F32, tag="kb")
            nc.sync.dma_start(kb[:], d_kb[:])
        if has_vb:
            vbr = cpool.tile([128, feat], F32, tag="vbr")
            nc.sync.dma_start(vbr[:], d_vbr[:])
        if has_mask:
            mvst = cpool.tile([128, n_st], F32, tag="mvst")
            nc.sync.dma_start(mvst[:], d_mvst[:])

        # big persistent tensors
        Kt = bigp.tile([128, pairs * s_len], F16, tag="kt")
        Vx = bigp.tile([128, (s_len // 128) * vxw], F16, tag="vx")
        AOt = bigp.tile([128, pairs * s_len], F32R, tag="aot")

        # Vx denominator/mask columns
        for st in range(n_st):
            dst = Vx[:, st * vxw:(st + 1) * vxw].rearrange(
                "p (h c) -> p h c", c=65)[:, :, 64:65]
            src = d_mvr[st * 128:(st + 1) * 128, :].rearrange(
                "p (h c) -> p h c", c=1)
            nc.sync.dma_start(dst, src)

        pvtags = ["pvA", "pvB"]

        def emit_kv_window(sw):
            yts = []
            for ct in range(n_ct):
                t = sbuf.tile([128, 512], F16, tag="s")
                nc.sync.dma_start(
                    t[:], d_yT[ct * 128:(ct + 1) * 128, sw * 512:(sw + 1) * 512])
                yts.append(t)
            for mt in range(pairs):
                pk = ps_pv.tile([128, 512], F32, tag=pvtags[mt % 2])
                for ct in range(n_ct):
                    nc.tensor.matmul(
                        pk[:],
                        wk[:, ct * feat + mt * 128: ct * feat + (mt + 1) * 128],
                        yts[ct][:],
                        start=(ct == 0), stop=(ct == n_ct - 1))
                dst = Kt[:, mt * s_len + sw * 512: mt * s_len + (sw + 1) * 512]
                if has_qkb:
                    nc.vector.tensor_scalar_add(dst, pk[:], kb[:, mt:mt + 1])
                else:
                    nc.vector.tensor_copy(dst, pk[:])
            for i in range(4):
                st = sw * 4 + i
                pvp = ps_pv.tile([128, feat], F32, tag=pvtags[i % 2])
                for ct in range(n_ct):
                    nc.tensor.matmul(
                        pvp[:], yts[ct][:, i * 128:(i + 1) * 128],
                        wv[:, ct * feat:(ct + 1) * feat],
                        start=(ct == 0), stop=(ct == n_ct - 1))
                dst = Vx[:, st * vxw:(st + 1) * vxw].rearrange(
                    "p (h c) -> p h c", c=65)[:, :, 0:64]
                src = pvp[:].rearrange("p (h c) -> p h c", c=64)
                if has_vb:
                    nc.vector.scalar_tensor_tensor(
                        out=dst, in0=src, scalar=0.0, in1=vbr[:].rearrange(
                            "p (h c) -> p h c", c=64),
                        op0=OP.bypass, op1=OP.add)
                    if has_mask:
                        nc.vector.tensor_scalar_mul(dst, dst, mvst[:, st:st + 1])
                elif has_mask:
                    nc.vector.tensor_scalar_mul(dst, src, mvst[:, st:st + 1])
                else:
                    nc.vector.tensor_copy(dst, src)

        xts_by_w = {}

        def emit_x_loads(w):
            if w in xts_by_w:
                return
            xts = []
            for ct in range(n_ct):
                t = sbuf.tile([128, 512], F16, tag="s")
                nc.sync.dma_start(
                    t[:], d_xT[ct * 128:(ct + 1) * 128, w * 512:(w + 1) * 512])
                xts.append(t)
            xts_by_w[w] = xts

        qtw_by_unit = {}

        def emit_qproj(w, p):
            emit_x_loads(w)
            xts = xts_by_w[w]
            pq = ps_s.tile([128, 512], F32, tag="sA")
            for ct in range(n_ct):
                nc.tensor.matmul(
                    pq[:],
                    wq[:, ct * feat + p * 128: ct * feat + (p + 1) * 128],
                    xts[ct][:],
                    start=(ct == 0), stop=(ct == n_ct - 1))
            Qtw = qtwp.tile([128, 512], F16, tag="q")
            if has_qkb:
                nc.vector.tensor_scalar_add(Qtw[:], pq[:], qb[:, p:p + 1])
            else:
                nc.vector.tensor_copy(Qtw[:], pq[:])
            qtw_by_unit[(w, p)] = Qtw

        def emit_oproj_block(st):
            co = 0
            for cwi, cw in enumerate(cw_sizes):
                po = ps_pv.tile([128, 512], F32, tag=pvtags[cwi % 2])
                for p4 in range(pairs):
                    nc.tensor.matmul(
                        po[:, 0:cw],
                        AOt[:, p4 * s_len + st * 128: p4 * s_len + (st + 1) * 128],
                        wo[:, p4 * cin + co: p4 * cin + co + cw],
                        start=(p4 == 0), stop=(p4 == pairs - 1))
                ot = rpool.tile([128, 512], F32, tag="osb")
                nc.vector.tensor_copy(ot[:, 0:cw], po[:, 0:cw])
                nc.sync.dma_start(
                    d_out[st * 128:(st + 1) * 128, co:co + cw], ot[:, 0:cw])
                co += cw

        units = [(w, p) for w in range(n_sw) for p in range(pairs)]

        def emit_unit(ui):
            w, p = units[ui]
            q0 = w * 512
            nxt = units[ui + 1] if ui + 1 < len(units) else None
            # O-projection blocks of the previous window, spread across units
            osts = []
            if w >= 1:
                lo, hi = p * 4 // pairs, (p + 1) * 4 // pairs
                osts = [4 * (w - 1) + i for i in range(lo, hi)]
            Qtw = qtw_by_unit.pop((w, p))
            nkt = 4 * (w + 1)
            pvA = ps_pv.tile([65, 512], F32, tag="pvA")
            pvB = ps_pv.tile([65, 512], F32, tag="pvB")
            pend = None  # (kt, c0, Pt) awaiting its PV matmuls

            def emit_pv(pkt, pc0, pPt, stop):
                nc.tensor.matmul(
                    pvA[:, pc0:512],
                    Vx[:, pkt * vxw + (2 * p) * 65: pkt * vxw + (2 * p) * 65 + 65],
                    pPt[:, pc0:512],
                    start=(pkt == 0), stop=stop)
                nc.tensor.matmul(
                    pvB[:, pc0:512],
                    Vx[:, pkt * vxw + (2 * p + 1) * 65: pkt * vxw + (2 * p + 1) * 65 + 65],
                    pPt[:, 512 + pc0:1024],
                    start=(pkt == 0), stop=stop)

            for kt in range(nkt):
                k0 = kt * 128
                diag = kt >= 4 * w
                c0 = (kt - 4 * w) * 128 if diag else 0
                sAB = ps_s.tile([128, 1024], F32, tag="sA")
                nc.tensor.matmul(
                    sAB[:, c0:512],
                    Kt[0:64, p * s_len + k0: p * s_len + k0 + 128],
                    Qtw[0:64, c0:512],
                    start=True, stop=True, tile_position=(0, 0))
                nc.tensor.matmul(
                    sAB[:, 512 + c0:1024],
                    Kt[64:128, p * s_len + k0: p * s_len + k0 + 128],
                    Qtw[64:128, c0:512],
                    start=True, stop=True, tile_position=(64, 0))
                Pt = ppool.tile([128, 1024], F16, tag="Pt")
                nc.scalar.activation(Pt[:, c0:1024], sAB[:, c0:1024],
                                     AF.Exp, scale=0.125)
                if diag:
                    nc.vector.tensor_tensor(
                        out=Pt[:, c0:c0 + 128], in0=Pt[:, c0:c0 + 128],
                        in1=tri[:], op=OP.mult)
                    nc.vector.tensor_tensor(
                        out=Pt[:, 512 + c0:512 + c0 + 128],
                        in0=Pt[:, 512 + c0:512 + c0 + 128],
                        in1=tri[:], op=OP.mult)
                if kt == 1 and nxt is not None:
                    emit_qproj(*nxt)  # pipeline next unit's Q one unit ahead
                if kt == 3:
                    for st in osts:
                        emit_oproj_block(st)
                if pend is not None:
                    emit_pv(*pend, stop=False)
                pend = (kt, c0, Pt)
            emit_pv(*pend, stop=True)
            for idx, pv in ((0, pvA), (1, pvB)):
                drow = dpool.tile([1, 512], F32R, tag=f"drow{idx}")
                nc.vector.tensor_copy(drow[:], pv[64:65, :])
                psD = ps_s.tile([64, 512], F32, tag="sA")
                nc.tensor.matmul(psD[:], ones1[:], drow[:],
                                 start=True, stop=True)
                rsb = rpool.tile([64, 512], F32, tag=f"rsb{idx}")
                nc.vector.reciprocal_approx_fast(out=rsb[:], in_=psD[:])
                nc.vector.tensor_tensor(
                    out=AOt[64 * idx: 64 * idx + 64,
                            p * s_len + q0: p * s_len + q0 + 512],
                    in0=pv[0:64, :], in1=rsb[:], op=OP.mult)

        # ---- emission schedule: overlap K/V windows with early attention ----
        emit_kv_window(0)
        if n_sw > 1:
            emit_kv_window(1)
        # O weights take a freed slot later; DMA can be emitted now
        wo = wpool.tile([128, pairs * cin], F32R, tag="w")
        for p4 in range(pairs):
            nc.sync.dma_start(wo[:, p4 * cin:(p4 + 1) * cin],
                              d_wo[p4 * 128:(p4 + 1) * 128, :])
        emit_qproj(0, 0)
        remaining_kv = list(range(2, n_sw))
        for ui in range(len(units)):
            if remaining_kv and ui < pairs:
                emit_kv_window(remaining_kv.pop(0))
            emit_unit(ui)
        for st in range(4 * (n_sw - 1), 4 * n_sw):   # last window's O-projection
            emit_oproj_block(st)

    nc.compile()
    return nc


_programs = {}


def _get_program(key):
    if key not in _programs:
        _programs[key] = build_program(S, C, 4, HD, *key)
    return _programs[key]


def make_core_inputs(x, y, mask, Wq_w, Wq_b, Wkv_w, Wkv_b, Wo_w,
                     s_len=S, cin=C, pairs=4, nh=NH):
    """Build the list of 8 per-core input dicts (host-side shard + permute)."""
    n_h = 2 * pairs
    feat = 128 * pairs
    has_mask = bool(np.any(mask))
    has_qkb = bool(np.any(Wq_b)) or bool(np.any(Wkv_b[:cin]))
    has_vb = bool(np.any(Wkv_b[cin:]))
    tri = np.triu(np.ones((128, 128), dtype=np.float32))
    ones1 = np.ones((1, 64), dtype=np.float32)
    n_st = s_len // 128

    in_maps = []
    for core in range(N_CORES):
        b = core // 2
        g = core % 2
        # feature permutation: col = 128*p + 64*half + d  <-  local head 2p+half
        cidx = np.arange(feat)
        pair_i = cidx // 128
        half = (cidx % 128) // 64
        d = cidx % 64
        qk_rows = (n_h * g + 2 * pair_i + half) * 64 + d
        v_rows = cin + (n_h * g + cidx // 64) * 64 + cidx % 64

        mvec = 1.0 - mask[b].astype(np.float32)
        m = {
            "xT": np.ascontiguousarray(x[b].T).astype(np.float16),
            "yT": np.ascontiguousarray(y[b].T).astype(np.float16),
            "wqT": np.ascontiguousarray(Wq_w[qk_rows, :].T).astype(np.float16),
            "wkT": np.ascontiguousarray(Wkv_w[qk_rows, :].T).astype(np.float16),
            "wvT": np.ascontiguousarray(Wkv_w[v_rows, :].T).astype(np.float16),
            "woA": _round_tf32(Wo_w[:, qk_rows].T),
            "tri": tri,
            "mvr": np.ascontiguousarray(
                np.repeat(mvec[:, None], n_h, axis=1)).astype(np.float16),
            "ones1": ones1,
        }
        if has_qkb:
            m["qb"] = np.ascontiguousarray(
                Wq_b[qk_rows].reshape(pairs, 128).T)
            m["kb"] = np.ascontiguousarray(
                Wkv_b[qk_rows].reshape(pairs, 128).T)
        if has_vb:
            m["vbr"] = np.tile(Wkv_b[v_rows][None, :], (128, 1))
        if has_mask:
            m["mvst"] = np.ascontiguousarray(
                mvec.reshape(n_st, 128).T)
        in_maps.append(m)
    return in_maps, (has_mask, has_qkb, has_vb)


def run(x, y, mask, Wq_w, Wq_b, Wkv_w, Wkv_b, Wo_w, Wo_b, trace=False):
    x = np.asarray(x, dtype=np.float32)
    y = np.asarray(y, dtype=np.float32)
    mask = np.asarray(mask)
    Wq_w = np.asarray(Wq_w, dtype=np.float32)
    Wq_b = np.asarray(Wq_b, dtype=np.float32)
    Wkv_w = np.asarray(Wkv_w, dtype=np.float32)
    Wkv_b = np.asarray(Wkv_b, dtype=np.float32)
    Wo_w = np.asarray(Wo_w, dtype=np.float32)
    Wo_b = np.asarray(Wo_b, dtype=np.float32)

    in_maps, flags = make_core_inputs(x, y, mask, Wq_w, Wq_b, Wkv_w, Wkv_b, Wo_w)
    nc = _get_program(flags)
    res = bass_utils.run_bass_kernel_spmd(
        nc, in_maps, core_ids=list(range(N_CORES)), trace=trace)
    out = np.empty((B, S, C), dtype=np.float32)
    for b in range(B):
        out[b] = res.results[2 * b]["out"] + res.results[2 * b + 1]["out"] + Wo_b
    return out, res


def kernel(x, y, mask, Wq_w, Wq_b, Wkv_w, Wkv_b, Wo_w, Wo_b):
    out, _ = run(x, y, mask, Wq_w, Wq_b, Wkv_w, Wkv_b, Wo_w, Wo_b, trace=False)
    return out


# revision 7
# speedup vs baseline: 1.6418x; 1.0802x over previous
"""Trainium2 Bass kernel for nn_CausalCrossAttention (B=4, S=2048, C=1024, NH=16, HD=64).

Sharding: DP over batch (4) x TP over heads (2 groups of 8), 8 NeuronCores.
Core c handles batch b = c // 2, head group g = c % 2 (heads 8g..8g+7).

Per-core algorithm (fp16 matmuls with fp32 PSUM accumulation; the final output
projection runs in float32r = tf32):
  - Host pre-transposes x/y to [C, S] and permutes weights head-pair-interleaved
    (pair p rows 128p..128p+127; even head partitions 0..63, odd head 64..127).
  - K-projection -> resident Kt [feat, S]; V-projection -> resident Vx [S, 8*65]
    with a padding-mask column per head (col 64 of each 65-wide head block) so
    the PV matmul also emits the softmax denominator as its output row 64.
  - Q-projection is done just-in-time per (q-window, pair) into a small tile.
  - Scores are computed transposed, S^T[k, q], two heads concurrently via PE
    row-groups (0,0)/(64,0) into ONE [128, 1024] two-bank PSUM tile; a single
    exp(s/8) activation covers both heads' score tiles and writes P^T (fp16).
    Causal: skip fully-masked k-tiles, shrink N for diagonal tiles, multiply
    the diagonal 128-col blocks by a constant triangular mask.
  - PV (software-pipelined one k-tile behind the scores so the PE never waits
    on the exp): psum[65, 512] += Vx_tile[k, 65].T @ P^T; row 64 = denominator.
  - Normalize: denom row -> SBUF (ScalarE copy), K=1 ones-matmul broadcasts it
    across 64 partitions in PSUM, approx-reciprocal on VectorE, fused multiply
    from PV psum into AOt [feat, S] (f32r, O-projection lhsT layout).
  - O-projection (interleaved after each q-window): out partial = AOt.T @ WoArr;
    host sums the two head-group partials and adds Wo_b.
"""

import numpy as np
from contextlib import ExitStack

import concourse.bacc as bacc
import concourse.mybir as mybir
import concourse.tile as tile
from concourse import bass_utils

F32 = mybir.dt.float32
F32R = mybir.dt.float32r
F16 = mybir.dt.float16
AF = mybir.ActivationFunctionType
OP = mybir.AluOpType

B, S, C, NH, HD = 4, 2048, 1024, 16, 64
N_CORES = 8


def _round_tf32(a: np.ndarray) -> np.ndarray:
    """Round fp32 -> tf32 (10-bit mantissa) with round-to-nearest-even."""
    a = np.ascontiguousarray(a, dtype=np.float32)
    u = a.view(np.uint32)
    lsb = (u >> np.uint32(13)) & np.uint32(1)
    r = (u + np.uint32(0x0FFF) + lsb) & np.uint32(0xFFFFE000)
    return r.view(np.float32)


def build_program(s_len, cin, pairs, hd=64, has_mask=False, has_qkb=False, has_vb=False):
    """Build + compile the per-core Bass program. Returns the Bacc object."""
    assert s_len % 512 == 0 and cin % 128 == 0 and hd == 64
    n_sw = s_len // 512          # 512-wide s windows
    n_ct = cin // 128            # cin contraction tiles
    n_st = s_len // 128          # 128-wide s tiles
    feat = 128 * pairs           # local feature dim (= 64 * local heads)
    n_h = 2 * pairs              # local heads
    vxw = n_h * 65               # Vx row width per k-tile
    cw_sizes = []
    rem = cin
    while rem > 0:
        cw_sizes.append(min(512, rem))
        rem -= 512

    nc = bacc.Bacc("TRN2", target_bir_lowering=False, debug=False,
                   num_devices=N_CORES)

    d_xT = nc.dram_tensor("xT", [cin, s_len], F16, kind="ExternalInput")
    d_yT = nc.dram_tensor("yT", [cin, s_len], F16, kind="ExternalInput")
    d_wq = nc.dram_tensor("wqT", [cin, feat], F16, kind="ExternalInput")
    d_wk = nc.dram_tensor("wkT", [cin, feat], F16, kind="ExternalInput")
    d_wv = nc.dram_tensor("wvT", [cin, feat], F16, kind="ExternalInput")
    d_wo = nc.dram_tensor("woA", [feat, cin], F32R, kind="ExternalInput")
    d_tri = nc.dram_tensor("tri", [128, 128], F32, kind="ExternalInput")
    d_mvr = nc.dram_tensor("mvr", [s_len, n_h], F16, kind="ExternalInput")
    d_ones = nc.dram_tensor("ones1", [1, 64], F32R, kind="ExternalInput")
    if has_qkb:
        d_qb = nc.dram_tensor("qb", [128, pairs], F32, kind="ExternalInput")
        d_kb = nc.dram_tensor("kb", [128, pairs], F32, kind="ExternalInput")
    if has_vb:
        d_vbr = nc.dram_tensor("vbr", [128, feat], F32, kind="ExternalInput")
    if has_mask:
        d_mvst = nc.dram_tensor("mvst", [128, n_st], F32, kind="ExternalInput")
    d_out = nc.dram_tensor("out", [s_len, cin], F32, kind="ExternalOutput")

    with tile.TileContext(nc) as tc, ExitStack() as ctx:
        cpool = ctx.enter_context(tc.tile_pool(name="const", bufs=1))
        wpool = ctx.enter_context(tc.tile_pool(name="wts", bufs=3))
        bigp = ctx.enter_context(tc.tile_pool(name="big", bufs=1))
        sbuf = ctx.enter_context(tc.tile_pool(name="stream", bufs=16))
        qtwp = ctx.enter_context(tc.tile_pool(name="qtw", bufs=3))
        ppool = ctx.enter_context(tc.tile_pool(name="pt", bufs=5))
        rpool = ctx.enter_context(tc.tile_pool(name="rp", bufs=2))
        dpool = ctx.enter_context(tc.tile_pool(name="dp", bufs=2))
        ps_pv = ctx.enter_context(tc.tile_pool(name="ps_pv", bufs=2, space="PSUM"))
        ps_s = ctx.enter_context(tc.tile_pool(name="ps_s", bufs=2, space="PSUM"))

        # K/V/Q weights first (their DMAs gate the first projections)
        wk = wpool.tile([128, n_ct * feat], F16, tag="w")
        wv = wpool.tile([128, n_ct * feat], F16, tag="w")
        wq = wpool.tile([128, n_ct * feat], F16, tag="w")
        for ct in range(n_ct):
            nc.sync.dma_start(wk[:, ct * feat:(ct + 1) * feat],
                              d_wk[ct * 128:(ct + 1) * 128, :])
        for ct in range(n_ct):
            nc.sync.dma_start(wv[:, ct * feat:(ct + 1) * feat],
                              d_wv[ct * 128:(ct + 1) * 128, :])
        for ct in range(n_ct):
            nc.sync.dma_start(wq[:, ct * feat:(ct + 1) * feat],
                              d_wq[ct * 128:(ct + 1) * 128, :])

        # constants
        tri = cpool.tile([128, 128], F32, tag="tri")
        nc.sync.dma_start(tri[:], d_tri[:])
        ones1 = cpool.tile([1, 64], F32R, tag="ones1")
        nc.sync.dma_start(ones1[:], d_ones[:])
        if has_qkb:
            qb = cpool.tile([128, pairs], F32, tag="qb")
            nc.sync.dma_start(qb[:], d_qb[:])
            kb = cpool.tile([128, pairs], F32, tag="kb")
            nc.sync.dma_start(kb[:], d_kb[:])
        if has_vb:
            vbr = cpool.tile([128, feat], F32, tag="vbr")
            nc.sync.dma_start(vbr[:], d_vbr[:])
        if has_mask:
            mvst = cpool.tile([128, n_st], F32, tag="mvst")
            nc.sync.dma_start(mvst[:], d_mvst[:])

        # big persistent tensors
        Kt = bigp.tile([128, pairs * s_len], F16, tag="kt")
        Vx = bigp.tile([128, (s_len // 128) * vxw], F16, tag="vx")
        AOt = bigp.tile([128, pairs * s_len], F32R, tag="aot")

        def emit_mvr_cols():
            for st in range(n_st):
                dst = Vx[:, st * vxw:(st + 1) * vxw].rearrange(
                    "p (h c) -> p h c", c=65)[:, :, 64:65]
                msrc = d_mvr[st * 128:(st + 1) * 128, :].rearrange(
                    "p (h c) -> p h c", c=1)
                nc.sync.dma_start(dst, msrc)

        pvtags = ["pvA", "pvB"]

        def emit_kv_window(sw):
            yts = []
            for ct in range(n_ct):
                t = sbuf.tile([128, 512], F16, tag="s")
                nc.sync.dma_start(
                    t[:], d_yT[ct * 128:(ct + 1) * 128, sw * 512:(sw + 1) * 512])
                yts.append(t)
            for mt in range(pairs):
                pk = ps_pv.tile([128, 512], F32, tag=pvtags[mt % 2])
                for ct in range(n_ct):
                    nc.tensor.matmul(
                        pk[:],
                        wk[:, ct * feat + mt * 128: ct * feat + (mt + 1) * 128],
                        yts[ct][:],
                        start=(ct == 0), stop=(ct == n_ct - 1))
                dst = Kt[:, mt * s_len + sw * 512: mt * s_len + (sw + 1) * 512]
                if has_qkb:
                    nc.vector.tensor_scalar_add(dst, pk[:], kb[:, mt:mt + 1])
                else:
                    nc.vector.tensor_copy(dst, pk[:])
            for i in range(4):
                st = sw * 4 + i
                pvp = ps_pv.tile([128, feat], F32, tag=pvtags[i % 2])
                for ct in range(n_ct):
                    nc.tensor.matmul(
                        pvp[:], yts[ct][:, i * 128:(i + 1) * 128],
                        wv[:, ct * feat:(ct + 1) * feat],
                        start=(ct == 0), stop=(ct == n_ct - 1))
                dst = Vx[:, st * vxw:(st + 1) * vxw].rearrange(
                    "p (h c) -> p h c", c=65)[:, :, 0:64]
                src = pvp[:].rearrange("p (h c) -> p h c", c=64)
                if has_vb:
                    nc.vector.scalar_tensor_tensor(
                        out=dst, in0=src, scalar=0.0, in1=vbr[:].rearrange(
                            "p (h c) -> p h c", c=64),
                        op0=OP.bypass, op1=OP.add)
                    if has_mask:
                        nc.vector.tensor_scalar_mul(dst, dst, mvst[:, st:st + 1])
                elif has_mask:
                    nc.vector.tensor_scalar_mul(dst, src, mvst[:, st:st + 1])
                else:
                    nc.vector.tensor_copy(dst, src)

        xts_by_w = {}

        def emit_x_loads(w):
            if w in xts_by_w:
                return
            xts = []
            for ct in range(n_ct):
                t = sbuf.tile([128, 512], F16, tag="s")
                nc.sync.dma_start(
                    t[:], d_xT[ct * 128:(ct + 1) * 128, w * 512:(w + 1) * 512])
                xts.append(t)
            xts_by_w[w] = xts

        qtw_by_unit = {}

        def emit_qproj(w, p):
            emit_x_loads(w)
            xts = xts_by_w[w]
            pq = ps_s.tile([128, 512], F32, tag="sA")
            for ct in range(n_ct):
                nc.tensor.matmul(
                    pq[:],
                    wq[:, ct * feat + p * 128: ct * feat + (p + 1) * 128],
                    xts[ct][:],
                    start=(ct == 0), stop=(ct == n_ct - 1))
            Qtw = qtwp.tile([128, 512], F16, tag="q")
            if has_qkb:
                nc.vector.tensor_scalar_add(Qtw[:], pq[:], qb[:, p:p + 1])
            else:
                nc.vector.tensor_copy(Qtw[:], pq[:])
            qtw_by_unit[(w, p)] = Qtw

        def emit_oproj_block(st):
            co = 0
            for cwi, cw in enumerate(cw_sizes):
                po = ps_pv.tile([128, 512], F32, tag=pvtags[cwi % 2])
                for p4 in range(pairs):
                    nc.tensor.matmul(
                        po[:, 0:cw],
                        AOt[:, p4 * s_len + st * 128: p4 * s_len + (st + 1) * 128],
                        wo[:, p4 * cin + co: p4 * cin + co + cw],
                        start=(p4 == 0), stop=(p4 == pairs - 1))
                ot = rpool.tile([128, 512], F32, tag="osb")
                nc.vector.tensor_copy(ot[:, 0:cw], po[:, 0:cw])
                nc.sync.dma_start(
                    d_out[st * 128:(st + 1) * 128, co:co + cw], ot[:, 0:cw])
                co += cw

        units = [(w, p) for w in range(n_sw) for p in range(pairs)]

        def emit_unit(ui):
            w, p = units[ui]
            q0 = w * 512
            nxt = units[ui + 1] if ui + 1 < len(units) else None
            # O-projection blocks of the previous window, spread across units
            osts = []
            if w >= 1:
                lo, hi = p * 4 // pairs, (p + 1) * 4 // pairs
                osts = [4 * (w - 1) + i for i in range(lo, hi)]
            Qtw = qtw_by_unit.pop((w, p))
            nkt = 4 * (w + 1)
            pvA = ps_pv.tile([65, 512], F32, tag="pvA")
            pvB = ps_pv.tile([65, 512], F32, tag="pvB")
            pend = []  # [(kt, c0, Pt)] awaiting PV matmuls (depth 2)

            def emit_pv(pkt, pc0, pPt, stop):
                nc.tensor.matmul(
                    pvA[:, pc0:512],
                    Vx[:, pkt * vxw + (2 * p) * 65: pkt * vxw + (2 * p) * 65 + 65],
                    pPt[:, pc0:512],
                    start=(pkt == 0), stop=stop)
                nc.tensor.matmul(
                    pvB[:, pc0:512],
                    Vx[:, pkt * vxw + (2 * p + 1) * 65: pkt * vxw + (2 * p + 1) * 65 + 65],
                    pPt[:, 512 + pc0:1024],
                    start=(pkt == 0), stop=stop)

            for kt in range(nkt):
                k0 = kt * 128
                diag = kt >= 4 * w
                c0 = (kt - 4 * w) * 128 if diag else 0
                sAB = ps_s.tile([128, 1024], F32, tag="sA")
                nc.tensor.matmul(
                    sAB[:, c0:512],
                    Kt[0:64, p * s_len + k0: p * s_len + k0 + 128],
                    Qtw[0:64, c0:512],
                    start=True, stop=True, tile_position=(0, 0))
                nc.tensor.matmul(
                    sAB[:, 512 + c0:1024],
                    Kt[64:128, p * s_len + k0: p * s_len + k0 + 128],
                    Qtw[64:128, c0:512],
                    start=True, stop=True, tile_position=(64, 0))
                Pt = ppool.tile([128, 1024], F16, tag="Pt")
                nc.scalar.activation(Pt[:, c0:1024], sAB[:, c0:1024],
                                     AF.Exp, scale=0.125)
                if diag:
                    nc.vector.tensor_tensor(
                        out=Pt[:, c0:c0 + 128], in0=Pt[:, c0:c0 + 128],
                        in1=tri[:], op=OP.mult)
                    nc.vector.tensor_tensor(
                        out=Pt[:, 512 + c0:512 + c0 + 128],
                        in0=Pt[:, 512 + c0:512 + c0 + 128],
                        in1=tri[:], op=OP.mult)
                if kt == 1 and nxt is not None:
                    emit_qproj(*nxt)  # pipeline next unit's Q one unit ahead
                if kt == 3:
                    for st in osts:
                        emit_oproj_block(st)
                if len(pend) >= 2:
                    emit_pv(*pend.pop(0), stop=False)
                pend.append((kt, c0, Pt))
            while pend:
                emit_pv(*pend.pop(0), stop=(not pend))
            for idx, pv in ((0, pvA), (1, pvB)):
                drow = dpool.tile([1, 512], F32R, tag=f"drow{idx}")
                nc.vector.tensor_copy(drow[:], pv[64:65, :])
                psD = ps_s.tile([64, 512], F32, tag="sA")
                nc.tensor.matmul(psD[:], ones1[:], drow[:],
                                 start=True, stop=True)
                rsb = rpool.tile([64, 512], F32, tag=f"rsb{idx}")
                nc.vector.reciprocal_approx_fast(out=rsb[:], in_=psD[:])
                nc.vector.tensor_tensor(
                    out=AOt[64 * idx: 64 * idx + 64,
                            p * s_len + q0: p * s_len + q0 + 512],
                    in0=pv[0:64, :], in1=rsb[:], op=OP.mult)

        # ---- emission schedule: overlap K/V windows with early attention ----
        emit_kv_window(0)
        emit_mvr_cols()
        # O weights take a freed slot later; DMA can be emitted now
        wo = wpool.tile([128, pairs * cin], F32R, tag="w")
        for p4 in range(pairs):
            nc.sync.dma_start(wo[:, p4 * cin:(p4 + 1) * cin],
                              d_wo[p4 * 128:(p4 + 1) * 128, :])
        emit_qproj(0, 0)
        remaining_kv = list(range(1, n_sw))
        for ui in range(len(units)):
            emit_unit(ui)
            if remaining_kv:
                emit_kv_window(remaining_kv.pop(0))
        for st in range(4 * (n_sw - 1), 4 * n_sw):   # last window's O-projection
            emit_oproj_block(st)

    nc.compile()
    return nc


_programs = {}


def _get_program(key):
    if key not in _programs:
        _programs[key] = build_program(S, C, 4, HD, *key)
    return _programs[key]


def make_core_inputs(x, y, mask, Wq_w, Wq_b, Wkv_w, Wkv_b, Wo_w,
                     s_len=S, cin=C, pairs=4, nh=NH):
    """Build the list of 8 per-core input dicts (host-side shard + permute)."""
    n_h = 2 * pairs
    feat = 128 * pairs
    has_mask = bool(np.any(mask))
    has_qkb = bool(np.any(Wq_b)) or bool(np.any(Wkv_b[:cin]))
    has_vb = bool(np.any(Wkv_b[cin:]))
    tri = np.triu(np.ones((128, 128), dtype=np.float32))
    ones1 = np.ones((1, 64), dtype=np.float32)
    n_st = s_len // 128

    in_maps = []
    for core in range(N_CORES):
        b = core // 2
        g = core % 2
        # feature permutation: col = 128*p + 64*half + d  <-  local head 2p+half
        cidx = np.arange(feat)
        pair_i = cidx // 128
        half = (cidx % 128) // 64
        d = cidx % 64
        qk_rows = (n_h * g + 2 * pair_i + half) * 64 + d
        v_rows = cin + (n_h * g + cidx // 64) * 64 + cidx % 64

        mvec = 1.0 - mask[b].astype(np.float32)
        m = {
            "xT": np.ascontiguousarray(x[b].T).astype(np.float16),
            "yT": np.ascontiguousarray(y[b].T).astype(np.float16),
            "wqT": np.ascontiguousarray(Wq_w[qk_rows, :].T).astype(np.float16),
            "wkT": np.ascontiguousarray(Wkv_w[qk_rows, :].T).astype(np.float16),
            "wvT": np.ascontiguousarray(Wkv_w[v_rows, :].T).astype(np.float16),
            "woA": _round_tf32(Wo_w[:, qk_rows].T),
            "tri": tri,
            "mvr": np.ascontiguousarray(
                np.repeat(mvec[:, None], n_h, axis=1)).astype(np.float16),
            "ones1": ones1,
        }
        if has_qkb:
            m["qb"] = np.ascontiguousarray(
                Wq_b[qk_rows].reshape(pairs, 128).T)
            m["kb"] = np.ascontiguousarray(
                Wkv_b[qk_rows].reshape(pairs, 128).T)
        if has_vb:
            m["vbr"] = np.tile(Wkv_b[v_rows][None, :], (128, 1))
        if has_mask:
            m["mvst"] = np.ascontiguousarray(
                mvec.reshape(n_st, 128).T)
        in_maps.append(m)
    return in_maps, (has_mask, has_qkb, has_vb)


def run(x, y, mask, Wq_w, Wq_b, Wkv_w, Wkv_b, Wo_w, Wo_b, trace=False):
    x = np.asarray(x, dtype=np.float32)
    y = np.asarray(y, dtype=np.float32)
    mask = np.asarray(mask)
    Wq_w = np.asarray(Wq_w, dtype=np.float32)
    Wq_b = np.asarray(Wq_b, dtype=np.float32)
    Wkv_w = np.asarray(Wkv_w, dtype=np.float32)
    Wkv_b = np.asarray(Wkv_b, dtype=np.float32)
    Wo_w = np.asarray(Wo_w, dtype=np.float32)
    Wo_b = np.asarray(Wo_b, dtype=np.float32)

    in_maps, flags = make_core_inputs(x, y, mask, Wq_w, Wq_b, Wkv_w, Wkv_b, Wo_w)
    nc = _get_program(flags)
    res = bass_utils.run_bass_kernel_spmd(
        nc, in_maps, core_ids=list(range(N_CORES)), trace=trace)
    out = np.empty((B, S, C), dtype=np.float32)
    for b in range(B):
        out[b] = res.results[2 * b]["out"] + res.results[2 * b + 1]["out"] + Wo_b
    return out, res


def kernel(x, y, mask, Wq_w, Wq_b, Wkv_w, Wkv_b, Wo_w, Wo_b):
    out, _ = run(x, y, mask, Wq_w, Wq_b, Wkv_w, Wkv_b, Wo_w, Wo_b, trace=False)
    return out
